# Initial kernel scaffold
#
"""Masked transformer encoder layer on 8 trn2 NeuronCores.

Sharding: pure data-parallel — batch B=8, one batch element per core, zero
collectives.  Each core runs the full layer on (N=1024, D=1024, H=16, F=4096).

Per-core pipeline (bf16 matmuls, fp32 accumulation / layernorm math):
  LN0 (token-major, bn_stats) -> h bf16 -> PE-transpose -> h^T (feature-major)
  q^T,k^T = Wqkv(q,k) @ h^T   (feature-major out)
  v       = h^T.T @ Wqkv(v)   (token-major out, +ones column for denominators)
  S^T     = k^T.T @ q^T   per head   (K=64, auto row-group packing)
  P^T     = exp(0.125*S^T + key_mask_bias)   (ACT, per-partition bias)
  out^T   = v_aug.T @ P^T  (row 64 = softmax denominator)
  attn^T  = out^T * bcast(1/denom)
  A       = attn^T.T @ Wproj^T (token-major) ; x1 = src + (1-mq)*w + mq*(A+bproj)
            (w = Wproj @ mean_j(v) + bproj handles fully-masked query rows)
  LN1 in-place (x1 -> x1n fp32) -> transpose -> x1n^T
  z^T     = W1 @ x1n^T ; gelu(+b1) ; y^T = W2 @ z^T
  out     = x1n + y^T.T + b2
"""

import numpy as np
import ml_dtypes

import concourse.bass as bass
import concourse.tile as tile
from concourse import bacc
from concourse import mybir
from concourse.bass_utils import run_bass_kernel_spmd

B, N, D, H, F = 8, 1024, 1024, 16, 4096
HD = D // H          # 64
P = 128
FC = D // P          # 8 feature chunks of D
TT = N // P          # 8 token tiles
GC = F // P          # 32 chunks of F
NEG = -1e30
EPS = 1e-5

f32 = mybir.dt.float32
bf16 = mybir.dt.bfloat16
AF = mybir.ActivationFunctionType
OP = mybir.AluOpType


def _layernorm_inplace_stats(nc, pools, x_ap):
    """Return (mean, rstd) APs ([128,1] each) for x_ap [128, 1024] fp32."""
    stats = pools["stats"].tile([P, 2, 6], f32)
    for sg in range(2):
        nc.vector.bn_stats(out=stats[:, sg, :], in_=x_ap[:, sg * 512:(sg + 1) * 512])
    mv = pools["mv"].tile([P, 2], f32)
    nc.vector.bn_aggr(out=mv[:], in_=stats[:])
    # rstd = 1/sqrt(var + eps)
    nc.scalar.activation(out=mv[:, 1:2], in_=mv[:, 1:2], func=AF.Sqrt,
                         bias=pools["eps"][:], scale=1.0)
    nc.vector.reciprocal(out=mv[:, 1:2], in_=mv[:, 1:2])
    return mv[:, 0:1], mv[:, 1:2]


def build_bass():
    nc = bacc.Bacc("TRN2")

    # ---------------- DRAM I/O ----------------
    src_h = nc.dram_tensor("src", [N, D], f32, kind="ExternalInput")
    kb_h = nc.dram_tensor("kbias", [TT, P], f32, kind="ExternalInput")
    mq_h = nc.dram_tensor("mq", [TT, P], f32, kind="ExternalInput")
    vecs_h = nc.dram_tensor("vecs", [6, D], f32, kind="ExternalInput")
    b1_h = nc.dram_tensor("b1r", [GC, P], f32, kind="ExternalInput")
    wqkv_h = nc.dram_tensor("wqkvT", [FC, P, 3 * D], bf16, kind="ExternalInput")
    wproj_h = nc.dram_tensor("wprojT", [FC, P, D], bf16, kind="ExternalInput")
    w1_h = nc.dram_tensor("w1T", [FC, P, F], bf16, kind="ExternalInput")
    w2_h = nc.dram_tensor("w2T", [GC, P, D], bf16, kind="ExternalInput")
    out_h = nc.dram_tensor("out", [N, D], f32, kind="ExternalOutput")

    with TileKernel(nc) as tk:
        tk.run(src_h, kb_h, mq_h, vecs_h, b1_h, wqkv_h, wproj_h, w1_h, w2_h, out_h)
    nc.compile()
    return nc


class TileKernel:
    def __init__(self, nc):
        self.nc = nc
        self.tc = tile.TileContext(nc)

    def __enter__(self):
        self.tc.__enter__()
        return self

    def __exit__(self, *a):
        return self.tc.__exit__(*a)

    def run(self, src_h, kb_h, mq_h, vecs_h, b1_h, wqkv_h, wproj_h, w1_h, w2_h, out_h):
        nc, tc = self.nc, self.tc
        from contextlib import ExitStack

        with ExitStack() as top:
            consts = top.enter_context(tc.tile_pool(name="consts", bufs=1))
            persist = top.enter_context(tc.tile_pool(name="persist", bufs=1))
            tmp_pool = top.enter_context(tc.tile_pool(name="tmp", bufs=2))
            stats_pool = top.enter_context(tc.tile_pool(name="stats", bufs=3))
            mv_pool = top.enter_context(tc.tile_pool(name="mv", bufs=4))
            
            # ---------- constants ----------
            ident = consts.tile([P, P], bf16)
            from concourse.masks import make_identity
            make_identity(nc, ident[:])
            ones_row = consts.tile([1, P], f32)
            nc.vector.memset(ones_row[:], 1.0)
            ones_col = consts.tile([P, 1], bf16)
            nc.vector.memset(ones_col[:], 1.0)
            ones_row_bf = consts.tile([1, P], bf16)
            nc.vector.memset(ones_row_bf[:], 1.0)
            eps_sb = consts.tile([P, 1], f32)
            nc.vector.memset(eps_sb[:], EPS)
            pools = {"stats": stats_pool, "mv": mv_pool, "eps": eps_sb}

            # DMA order tuned for startup: g0/beta0 broadcasts, then src
            # (LN0 gates everything), then the rest of the small constants.
            bcast = consts.tile([P, 6, D], f32)

            def _bcast_dma(v6):
                bc_src = bass.AP(tensor=vecs_h[0:1, :].tensor, offset=v6 * D,
                                 ap=[[0, P], [1, D]])
                nc.sync.dma_start(out=bcast[:, v6, :], in_=bc_src)

            for v6 in (0, 1):
                _bcast_dma(v6)

            src_sb = persist.tile([P, TT, D], f32)   # src -> srcw -> x1 -> x1n
            for tt in range(TT):
                nc.sync.dma_start(out=src_sb[:, tt, :],
                                  in_=src_h[tt * P:(tt + 1) * P, :])

            kb_sb = consts.tile([P, TT], f32)
            nc.sync.dma_start(out=kb_sb[:], in_=kb_h[:, :].rearrange("a p -> p a"))
            mq_sb = consts.tile([P, TT], f32)
            nc.sync.dma_start(out=mq_sb[:], in_=mq_h[:, :].rearrange("a p -> p a"))
            invmq_sb = consts.tile([P, TT], f32)
            nc.vector.tensor_scalar(out=invmq_sb[:], in0=mq_sb[:], scalar1=-1.0,
                                    scalar2=1.0, op0=OP.mult, op1=OP.add)
            b1_sb = consts.tile([P, GC], f32)
            nc.sync.dma_start(out=b1_sb[:], in_=b1_h[:, :].rearrange("g p -> p g"))
            for v6 in (2, 3, 4, 5):
                _bcast_dma(v6)
            g0b, beta0b = bcast[:, 0], bcast[:, 1]
            g1b, beta1b = bcast[:, 2], bcast[:, 3]
            bprojb, b2b = bcast[:, 4], bcast[:, 5]

            wb_sb = consts.tile([P, D], f32)       # (Wproj @ mean_j v + bproj) broadcast
            u_sb = consts.tile([P, FC], bf16)      # mean_j v, feature-major columns

            with ExitStack() as attn_scope:
                qkT = attn_scope.enter_context(tc.tile_pool(name="qkT", bufs=1))
                vp = attn_scope.enter_context(tc.tile_pool(name="vsb", bufs=1))

                qkT_sb = qkT.tile([P, 16, N], bf16)
                v_sb = vp.tile([P, TT, H, HD + 1], bf16)
                nc.vector.memset(v_sb[:, :, :, HD:HD + 1], 1.0)

                # ================= LN0 + transpose + QKV =================
                with ExitStack() as qkv_scope:
                    htp = qkv_scope.enter_context(tc.tile_pool(name="hT", bufs=1))
                    hbp = qkv_scope.enter_context(tc.tile_pool(name="hbf", bufs=2))
                    trps = qkv_scope.enter_context(
                        tc.tile_pool(name="trps", bufs=2, space="PSUM"))
                    qkps = qkv_scope.enter_context(
                        tc.tile_pool(name="qkps", bufs=3, space="PSUM"))

                    hT_sb = htp.tile([P, FC, N], bf16)

                    for tt in range(TT):
                        x = src_sb[:, tt, :]
                        mean, rstd = _layernorm_inplace_stats(nc, pools, x)
                        ht = tmp_pool.tile([P, D], f32, tag="lnt")
                        nc.vector.tensor_scalar(out=ht[:], in0=x, scalar1=mean,
                                                scalar2=rstd, op0=OP.subtract, op1=OP.mult)
                        nc.gpsimd.tensor_tensor(ht[:], ht[:], g0b, OP.mult)
                        hbf = hbp.tile([P, D], bf16)
                        nc.vector.tensor_tensor(hbf[:], ht[:], beta0b, OP.add)
                        for fb in range(FC):
                            ps = trps.tile([P, P], bf16)
                            nc.tensor.transpose(ps[:], hbf[:, fb * P:(fb + 1) * P], ident[:])
                            nc.scalar.copy(hT_sb[:, fb, tt * P:(tt + 1) * P], ps[:])

                    # q^T, k^T (feature-major)
                    with tc.tile_pool(name="wqk", bufs=1) as wqkp:
                        wqk_sb = wqkp.tile([P, FC, 2 * D], bf16)
                        for fc in range(FC):
                            nc.sync.dma_start(out=wqk_sb[:, fc, :],
                                              in_=wqkv_h[fc, :, 0:2 * D])
                        for oc in range(16):
                            ps = qkps.tile([P, 1024], f32)
                            for ib in range(2):
                                for fc in range(FC):
                                    nc.tensor.matmul(
                                        ps[:, ib * 512:(ib + 1) * 512],
                                        wqk_sb[:, fc, oc * P:(oc + 1) * P],
                                        hT_sb[:, fc, ib * 512:(ib + 1) * 512],
                                        start=(fc == 0), stop=(fc == FC - 1))
                            nc.vector.tensor_copy(qkT_sb[:, oc, :], ps[:])

                    # v (token-major) into per-head lhsT layout
                    with tc.tile_pool(name="wv", bufs=1) as wvp:
                        wv_sb = wvp.tile([P, FC, D], bf16)
                        nc.sync.dma_start(out=wv_sb[:],
                                          in_=wqkv_h[:, :, 2 * D:3 * D].rearrange("f p o -> p f o"))
                        for tt in range(TT):
                            ps = qkps.tile([P, 1024], f32)
                            for vb in range(2):
                                for fc in range(FC):
                                    nc.tensor.matmul(
                                        ps[:, vb * 512:(vb + 1) * 512],
                                        hT_sb[:, fc, tt * P:(tt + 1) * P],
                                        wv_sb[:, fc, vb * 512:(vb + 1) * 512],
                                        start=(fc == 0), stop=(fc == FC - 1))
                            nc.vector.tensor_copy(
                                v_sb[:, tt, :, 0:HD],
                                ps[:].rearrange("p (h c) -> p h c", h=H))

                atp = attn_scope.enter_context(tc.tile_pool(name="attnT", bufs=1))
                wpp = attn_scope.enter_context(tc.tile_pool(name="wproj", bufs=1))
                attnT_sb = atp.tile([P, FC, N], bf16)
                wproj_sb = wpp.tile([P, FC, D], bf16)
                nc.sync.dma_start(out=wproj_sb[:],
                                  in_=wproj_h[:, :, :].rearrange("f p o -> p f o"))

                # ============ u = mean_j v ; w = Wproj @ u + bproj ============
                with tc.tile_pool(name="uwps", bufs=2, space="PSUM") as uwps, \
                        tc.tile_pool(name="wrowp", bufs=1) as wrowp:
                    wrow = wrowp.tile([1, D], f32)
                    for fc in range(FC):
                        ps = uwps.tile([P, 512], f32, tag="ups")
                        for hh in range(2):
                            for jc in range(TT):
                                nc.tensor.matmul(ps[hh * HD:(hh + 1) * HD, 0:1],
                                                 v_sb[:, jc, 2 * fc + hh, 0:HD],
                                                 ones_col[:],
                                                 start=(jc == 0), stop=(jc == TT - 1))
                        nc.vector.tensor_scalar(out=u_sb[:, fc:fc + 1], in0=ps[:, 0:1],
                                                scalar1=1.0 / N, scalar2=None, op0=OP.mult)
                    for ob in range(2):
                        ps = uwps.tile([P, 512], f32, tag="wps")
                        for fc in range(FC):
                            nc.tensor.matmul(ps[0:1, :], u_sb[:, fc:fc + 1],
                                             wproj_sb[:, fc, ob * 512:(ob + 1) * 512],
                                             start=(fc == 0), stop=(fc == FC - 1))
                        nc.vector.tensor_tensor(wrow[:, ob * 512:(ob + 1) * 512], ps[0:1, :],
                                                bprojb[0:1, ob * 512:(ob + 1) * 512], OP.add)
                    for ob in range(2):
                        ps = uwps.tile([P, 512], f32, tag="wbps")
                        nc.tensor.matmul(ps[:], ones_row[:],
                                         wrow[:, ob * 512:(ob + 1) * 512],
                                         start=True, stop=True)
                        nc.vector.tensor_copy(wb_sb[:, ob * 512:(ob + 1) * 512], ps[:])

                # srcw = src + (1-mq)*wb + mq*bprojb   (in place)
                for tt in range(TT):
                    t = tmp_pool.tile([P, D], f32, tag="srcw")
                    nc.vector.tensor_scalar(out=t[:], in0=wb_sb[:],
                                            scalar1=invmq_sb[:, tt:tt + 1],
                                            scalar2=None, op0=OP.mult)
                    nc.vector.tensor_tensor(src_sb[:, tt, :], src_sb[:, tt, :], t[:], OP.add)
                    t2 = tmp_pool.tile([P, D], f32, tag="srcw")
                    nc.gpsimd.tensor_scalar(out=t2[:], in0=bprojb[:],
                                            scalar1=mq_sb[:, tt:tt + 1],
                                            scalar2=None, op0=OP.mult)
                    nc.gpsimd.tensor_tensor(src_sb[:, tt, :], src_sb[:, tt, :], t2[:], OP.add)

                # ================= attention + proj =================
                with ExitStack() as att:
                    ptp = att.enter_context(tc.tile_pool(name="pt", bufs=12))
                    rdp = att.enter_context(tc.tile_pool(name="rd", bufs=3))
                    dnp = att.enter_context(tc.tile_pool(name="dn", bufs=3))
                    sps = att.enter_context(tc.tile_pool(name="sps", bufs=2, space="PSUM"))
                    avps = att.enter_context(tc.tile_pool(name="avps", bufs=3, space="PSUM"))
                    bcps2 = att.enter_context(tc.tile_pool(name="bcps2", bufs=1, space="PSUM"))

                    for h in range(H):
                        hp = (h % 2) * HD
                        fc_h = h // 2
                        # S^T for both i-blocks into one 2-bank psum; single exp
                        pts = []
                        for jc in range(TT):
                            ps_s = sps.tile([P, 1024], f32)
                            for ib in range(2):
                                nc.tensor.matmul(
                                    ps_s[:, ib * 512:(ib + 1) * 512],
                                    qkT_sb[hp:hp + HD, 8 + fc_h, jc * P:(jc + 1) * P],
                                    qkT_sb[hp:hp + HD, fc_h, ib * 512:(ib + 1) * 512],
                                    start=True, stop=True)
                            pt = ptp.tile([P, 1024], bf16)
                            nc.scalar.activation(out=pt[:], in_=ps_s[:], func=AF.Exp,
                                                 bias=kb_sb[:, jc:jc + 1], scale=0.125)
                            pts.append(pt)
                        for ib in range(2):
                            isl = slice(ib * 512, (ib + 1) * 512)
                            ps_av = avps.tile([P, 512], f32)
                            for jc in range(TT):
                                nc.tensor.matmul(ps_av[0:HD + 1, :],
                                                 v_sb[:, jc, h, :], pts[jc][:, isl],
                                                 start=(jc == 0), stop=(jc == TT - 1))
                            dn = dnp.tile([1, 512], bf16)
                            nc.vector.tensor_copy(dn[:], ps_av[HD:HD + 1, :])
                            ps_b = bcps2.tile([P, 512], f32)
                            nc.tensor.matmul(ps_b[:], ones_row_bf[:], dn[:],
                                             start=True, stop=True)
                            rd = rdp.tile([P, 512], f32)
                            nc.vector.reciprocal(rd[:], ps_b[:])
                            nc.vector.tensor_tensor(
                                attnT_sb[hp:hp + HD, fc_h, isl],
                                ps_av[0:HD, :], rd[0:HD, :], OP.mult)



                # proj + x1 (into src_sb); own psum scope after attention frees banks
                with tc.tile_pool(name="pps", bufs=3, space="PSUM") as pps:
                    for tt in range(TT):
                        for ob in range(2):
                            osl = slice(ob * 512, (ob + 1) * 512)
                            ps_p = pps.tile([P, 512], f32)
                            for fc in range(FC):
                                nc.tensor.matmul(ps_p[:],
                                                 attnT_sb[:, fc, tt * P:(tt + 1) * P],
                                                 wproj_sb[:, fc, osl],
                                                 start=(fc == 0), stop=(fc == FC - 1))
                            t = tmp_pool.tile([P, 512], f32, tag="x1t")
                            nc.vector.tensor_scalar(out=t[:], in0=ps_p[:],
                                                    scalar1=mq_sb[:, tt:tt + 1],
                                                    scalar2=None, op0=OP.mult)
                            nc.vector.tensor_tensor(src_sb[:, tt, osl],
                                                    src_sb[:, tt, osl], t[:], OP.add)

            # ================= LN1 (in place) + transpose =================
            with ExitStack() as ffn1:
                ztp = ffn1.enter_context(tc.tile_pool(name="zT", bufs=1))
                zT_sb = ztp.tile([P, GC, N], bf16)
                f1 = ffn1.enter_context(ExitStack())
                xtp = f1.enter_context(tc.tile_pool(name="x1nT", bufs=1))
                xbp = f1.enter_context(tc.tile_pool(name="x1nbf", bufs=2))
                w1p = f1.enter_context(tc.tile_pool(name="w1p", bufs=3))
                trps2 = f1.enter_context(tc.tile_pool(name="trps2", bufs=3, space="PSUM"))
                zps = f1.enter_context(tc.tile_pool(name="zps", bufs=2, space="PSUM"))

                x1nT_sb = xtp.tile([P, FC, N], bf16)

                for tt in range(TT):
                    x = src_sb[:, tt, :]
                    mean, rstd = _layernorm_inplace_stats(nc, pools, x)
                    nc.vector.tensor_scalar(out=x, in0=x, scalar1=mean,
                                            scalar2=rstd, op0=OP.subtract, op1=OP.mult)
                    nc.gpsimd.tensor_tensor(x, x, g1b, OP.mult)
                    nc.vector.tensor_tensor(x, x, beta1b, OP.add)
                    xbf = xbp.tile([P, D], bf16)
                    nc.gpsimd.tensor_copy(xbf[:], x)
                    for fb in range(FC):
                        ps = trps2.tile([P, P], bf16)
                        nc.tensor.transpose(ps[:], xbf[:, fb * P:(fb + 1) * P], ident[:])
                        nc.scalar.copy(x1nT_sb[:, fb, tt * P:(tt + 1) * P], ps[:])

                # ---------------- FFN linear1 + gelu ----------------
                for gc in range(GC):
                    w1t = w1p.tile([P, FC, P], bf16)
                    nc.sync.dma_start(out=w1t[:],
                                      in_=w1_h[:, :, gc * P:(gc + 1) * P].rearrange("f p c -> p f c"))
                    ps = zps.tile([P, 1024], f32)
                    for ib in range(2):
                        for fc in range(FC):
                            nc.tensor.matmul(ps[:, ib * 512:(ib + 1) * 512],
                                             w1t[:, fc, :],
                                             x1nT_sb[:, fc, ib * 512:(ib + 1) * 512],
                                             start=(fc == 0), stop=(fc == FC - 1))
                    nc.scalar.activation(out=zT_sb[:, gc, :], in_=ps[:], func=AF.Gelu,
                                         bias=b1_sb[:, gc:gc + 1], scale=1.0)

                # -------- FFN linear2, token-major y, fused residual+out --------
                f1.close()
                with ExitStack() as ffn2:
                    w2p = ffn2.enter_context(tc.tile_pool(name="w2p", bufs=2))
                    yout = ffn2.enter_context(tc.tile_pool(name="yout", bufs=4))
                    yps = ffn2.enter_context(tc.tile_pool(name="yps", bufs=3, space="PSUM"))

                    QW = 256
                    for ob in range(4):
                        osl = slice(ob * QW, (ob + 1) * QW)
                        w2q = w2p.tile([P, GC, QW], bf16)
                        nc.sync.dma_start(out=w2q[:],
                                          in_=w2_h[:, :, osl].rearrange("g p c -> p g c"))
                        for tt in range(TT):
                            ps = yps.tile([P, QW], f32)
                            for gc in range(GC):
                                nc.tensor.matmul(ps[:],
                                                 zT_sb[:, gc, tt * P:(tt + 1) * P],
                                                 w2q[:, gc, :],
                                                 start=(gc == 0), stop=(gc == GC - 1))
                            t = yout.tile([P, QW], f32)
                            nc.vector.tensor_tensor(t[:], ps[:], b2b[:, osl], OP.add)
                            nc.vector.tensor_tensor(t[:], t[:], src_sb[:, tt, osl], OP.add)
                            nc.sync.dma_start(out=out_h[tt * P:(tt + 1) * P, osl], in_=t[:])


_NC_CACHE = {}


def _get_nc():
    if "nc" not in _NC_CACHE:
        _NC_CACHE["nc"] = build_bass()
    return _NC_CACHE["nc"]


def prep_in_maps(inputs):
    src = np.asarray(inputs["src"], dtype=np.float32)          # [B, N, D]
    mask = np.asarray(inputs["mask"])                          # [B, N] bool
    Wqkv = np.asarray(inputs["Wqkv"], dtype=np.float32)
    Wproj = np.asarray(inputs["Wproj"], dtype=np.float32)
    bproj = np.asarray(inputs["bproj"], dtype=np.float32)
    W1 = np.asarray(inputs["W1"], dtype=np.float32)
    b1 = np.asarray(inputs["b1"], dtype=np.float32)
    W2 = np.asarray(inputs["W2"], dtype=np.float32)
    b2 = np.asarray(inputs["b2"], dtype=np.float32)
    g0 = np.asarray(inputs["g0"], dtype=np.float32)
    beta0 = np.asarray(inputs["beta0"], dtype=np.float32)
    g1 = np.asarray(inputs["g1"], dtype=np.float32)
    beta1 = np.asarray(inputs["beta1"], dtype=np.float32)

    bf = ml_dtypes.bfloat16
    wqkvT = np.ascontiguousarray(Wqkv.T).reshape(FC, P, 3 * D).astype(bf)
    wprojT = np.ascontiguousarray(Wproj.T).reshape(FC, P, D).astype(bf)
    w1T = np.ascontiguousarray(W1.T).reshape(FC, P, F).astype(bf)
    w2T = np.ascontiguousarray(W2.T).reshape(GC, P, D).astype(bf)
    vecs = np.ascontiguousarray(np.stack([g0, beta0, g1, beta1, bproj, b2]))
    b1r = np.ascontiguousarray(b1.reshape(GC, P))
    kbias = np.where(mask, 0.0, NEG).astype(np.float32).reshape(B, TT, P)
    mqf = mask.astype(np.float32).reshape(B, TT, P)

    in_maps = []
    for b in range(B):
        in_maps.append({
            "src": np.ascontiguousarray(src[b]),
            "kbias": np.ascontiguousarray(kbias[b]),
            "mq": np.ascontiguousarray(mqf[b]),
            "vecs": vecs,
            "b1r": b1r,
            "wqkvT": wqkvT,
            "wprojT": wprojT,
            "w1T": w1T,
            "w2T": w2T,
        })
    return in_maps


def kernel(**inputs):
    in_maps = prep_in_maps(inputs)
    nc = _get_nc()
    res = run_bass_kernel_spmd(nc, in_maps, core_ids=list(range(B)))
    return np.stack([r["out"] for r in res.results]).astype(np.float32)



# revision 3
# speedup vs baseline: 1.0187x; 1.0187x over previous
"""Masked transformer encoder layer on 8 trn2 NeuronCores, v2.

Data-parallel over batch (1 element/core). Per-core schedule built around the
TimelineSim cost model:
  - fp8e4 DoubleRow matmuls for QKV, S=q.k^T and P@V (0.5 cyc/output row).
  - bf16 matmuls for proj and the FFN (fp8 FFN would bust the 2e-2 gate).
  - token-major P@V (full 128-partition outputs) + per-partition normalize.
  - all transposes via the DMA xbar (dma_start_transpose), none on PE.
  - softmax key-masking by zeroing masked v rows + mask-valued denominator
    column (no per-key bias in exp, enabling wide exp instructions).
  - masked query rows replaced by wb = Wproj @ (Wv' @ mean(h)) + bproj'.
  - LN affine folded into weights on host; rsqrt via bit-trick + Newton on
    DVE so ACT only runs exp / gelu / fp8 converts.
  - query-halves A/B pipelined: FFN(A) overlaps exp(B) on ACT.
"""

import os
import numpy as np
import ml_dtypes

PAINT = 0.0

import concourse.bass as bass
import concourse.tile as tile
from concourse import bacc
from concourse import mybir
from concourse.bass_utils import run_bass_kernel_spmd

B, N, D, H, F = 8, 1024, 1024, 16, 4096
HD = D // H          # 64
P = 128
TT = N // P          # 8 token tiles
FC = D // P          # 8 feature chunks
GC = F // P          # 32 FFN1 chunks
EPS = 1e-5

f32 = mybir.dt.float32
bf16 = mybir.dt.bfloat16
fp8 = mybir.dt.float8e4
i32 = mybir.dt.int32
AF = mybir.ActivationFunctionType
OP = mybir.AluOpType
AX = mybir.AxisListType
DR = mybir.MatmulPerfMode.DoubleRow

MAGIC = 0x5F3759DF


def build_bass():
    nc = bacc.Bacc("TRN2")

    src_h = nc.dram_tensor("src", [N, D], f32, kind="ExternalInput")
    mq_h = nc.dram_tensor("mq", [TT, P], f32, kind="ExternalInput")
    imq_h = nc.dram_tensor("imq", [TT, P], f32, kind="ExternalInput")
    vecs_h = nc.dram_tensor("vecs", [3, D], f32, kind="ExternalInput")
    qkb_h = nc.dram_tensor("qkb", [16, P], f32, kind="ExternalInput")
    b1p_h = nc.dram_tensor("b1p", [GC, P], f32, kind="ExternalInput")
    wqk_h = nc.dram_tensor("wqk", [P, FC, 2 * D], fp8, kind="ExternalInput")
    wv_h = nc.dram_tensor("wv", [P, FC, D], fp8, kind="ExternalInput")
    wpj_h = nc.dram_tensor("wpj", [P, FC, D], bf16, kind="ExternalInput")
    w1_h = nc.dram_tensor("w1", [GC, P, D], bf16, kind="ExternalInput")
    w2_h = nc.dram_tensor("w2", [4, P, GC * 256], bf16, kind="ExternalInput")
    out_h = nc.dram_tensor("out", [N, D], f32, kind="ExternalOutput")

    with tile.TileContext(nc) as tc:
        Builder(nc, tc).run(src_h, mq_h, imq_h, vecs_h, qkb_h, b1p_h,
                            wqk_h, wv_h, wpj_h, w1_h, w2_h, out_h)
    nc.compile()
    return nc


class Builder:
    def __init__(self, nc, tc):
        self.nc = nc
        self.tc = tc

    # ---------- helpers ----------
    def rsqrt(self, pool, ve_ap, n):
        """(ve)^-0.5 elementwise for [128, n] f32 via bit trick + 2 Newton."""
        nc = self.nc
        t = pool.tile([P, n], i32, tag="rsq_t", name="rsq_t")
        nc.vector.tensor_scalar(out=t[:], in0=ve_ap.bitcast(i32), scalar1=1,
                                scalar2=None, op0=OP.arith_shift_right)
        y = pool.tile([P, n], f32, tag="rsq_y", name="rsq_y")
        nc.vector.tensor_scalar(out=y[:].bitcast(i32), in0=t[:], scalar1=-1,
                                scalar2=MAGIC, op0=OP.mult, op1=OP.add)
        for _ in range(2):
            a = pool.tile([P, n], f32, tag="rsq_a", name="rsq_a")
            nc.vector.tensor_tensor(a[:], y[:], y[:], OP.mult)
            nc.vector.tensor_tensor(a[:], a[:], ve_ap, OP.mult)
            nc.vector.tensor_scalar(out=a[:], in0=a[:], scalar1=-0.5,
                                    scalar2=1.5, op0=OP.mult, op1=OP.add)
            nc.vector.tensor_tensor(y[:], y[:], a[:], OP.mult)
        return y

    def ln_stats(self, x_ap):
        nc = self.nc
        st = self.stats_p.tile([P, 2, 6], f32, tag="bst", name="bst")
        for sg in range(2):
            nc.vector.bn_stats(out=st[:, sg, :],
                               in_=x_ap[:, sg * 512:(sg + 1) * 512])
        mv = self.mv_p.tile([P, 2], f32, tag="mv", name="mv")
        nc.vector.bn_aggr(out=mv[:], in_=st[:])
        return mv

    # ---------- main ----------
    def run(self, src_h, mq_h, imq_h, vecs_h, qkb_h, b1p_h,
            wqk_h, wv_h, wpj_h, w1_h, w2_h, out_h):
        nc, tc = self.nc, self.tc
        from contextlib import ExitStack

        with ExitStack() as left:
            consts = left.enter_context(tc.tile_pool(name="consts", bufs=1, side="left"))
            persist = left.enter_context(tc.tile_pool(name="persist", bufs=1, side="left"))
            small = left.enter_context(tc.tile_pool(name="small", bufs=1, side="left"))
            self.stats_p = left.enter_context(tc.tile_pool(name="stats", bufs=3, side="left"))
            self.mv_p = left.enter_context(tc.tile_pool(name="mv", bufs=12, side="left"))
            rs_p = left.enter_context(tc.tile_pool(name="rs", bufs=2, side="left"))
            tmp_p = left.enter_context(tc.tile_pool(name="tmp", bufs=3, side="left"))
            xbf_p = left.enter_context(tc.tile_pool(name="xbf", bufs=2, side="left"))
            attnT_stack = ExitStack()
            attnT_p = attnT_stack.enter_context(
                tc.tile_pool(name="attnTp", bufs=4, side="left"))
            atok_stack = ExitStack()
            atok_p = atok_stack.enter_context(
                tc.tile_pool(name="atokp", bufs=4, side="left"))
            pts_stack = ExitStack()
            pts_p = pts_stack.enter_context(
                tc.tile_pool(name="ptsp", bufs=5, side="left"))

            # ---------------- inputs (src first; weights after LN0) --------
            src_sb = persist.tile([P, TT, D], f32)
            for tt in range(TT):
                nc.sync.dma_start(out=src_sb[:, tt, :],
                                  in_=src_h[tt * P:(tt + 1) * P, :])
            mq_sb = consts.tile([P, TT], f32)
            imq_sb = consts.tile([P, TT], f32)
            qkb_sb = consts.tile([P, 16], f32)
            b1p_sb = consts.tile([P, GC], f32)
            bcast = consts.tile([P, 3, D], f32)
            g1b, bb2b, bprojb = bcast[:, 0], bcast[:, 1], bcast[:, 2]

            ones_row = consts.tile([1, P], bf16)
            nc.vector.memset(ones_row[:], 1.0)
            wb_sb = consts.tile([P, D], f32)
            hbar8 = consts.tile([P, FC], fp8)
            hbar_f = consts.tile([P, FC], f32)

            # right-side scoped pools: early weights + staging
            right1 = ExitStack()
            early = right1.enter_context(
                tc.tile_pool(name="early", bufs=1, side="right"))
            wqk_sb = early.tile([P, FC, 2 * D], fp8)
            wv_sb = early.tile([P, FC, D], fp8)
            hTb = early.tile([P, FC, N], bf16)
            hT = early.tile([P, FC, N], fp8)
            right2 = ExitStack()
            wpjA_p = right2.enter_context(
                tc.tile_pool(name="wpjA", bufs=1, side="right"))
            wproj_sb = wpjA_p.tile([P, FC, D], bf16)

            def late_input_dmas():
                nc.sync.dma_start(out=qkb_sb[:],
                                  in_=qkb_h[:, :].rearrange("a p -> p a"))
                nc.sync.dma_start(out=mq_sb[:],
                                  in_=mq_h[:, :].rearrange("a p -> p a"))
                for g in range(4):
                    nc.sync.dma_start(out=wqk_sb[:, 2 * g:2 * g + 2, :],
                                      in_=wqk_h[:, 2 * g:2 * g + 2, :])
                nc.sync.dma_start(out=wv_sb[:], in_=wv_h[:, :, :])
                nc.sync.dma_start(out=imq_sb[:],
                                  in_=imq_h[:, :].rearrange("a p -> p a"))
                nc.sync.dma_start(out=b1p_sb[:],
                                  in_=b1p_h[:, :].rearrange("a p -> p a"))
                for v3 in range(3):
                    bc_src = bass.AP(tensor=vecs_h[0:1, :].tensor, offset=v3 * D,
                                     ap=[[0, P], [1, D]])
                    nc.sync.dma_start(out=bcast[:, v3, :], in_=bc_src)
                nc.sync.dma_start(out=wproj_sb[:], in_=wpj_h[:, :, :])

            # attention-lifetime left pool (qkT, v) — closes before w2 opens
            attn_sb = ExitStack()
            attn_pool = attn_sb.enter_context(
                tc.tile_pool(name="attnsb", bufs=1, side="left"))
            qkT = attn_pool.tile([P, 16, N], fp8)
            v_sb = attn_pool.tile([P, TT, H, HD + 1], fp8)

            # ---------------- LN0 -> xbf -> hTb -> hT ----------------
            mv0 = []
            rstd0 = [None] * 2
            for tt in range(TT):
                mv0.append(self.ln_stats(src_sb[:, tt, :]))
                if tt % 4 == 3:
                    X4 = tt // 4
                    ve = rs_p.tile([P, 4], f32, tag="ve", name="ve0")
                    for k in range(4):
                        nc.vector.tensor_scalar(out=ve[:, k:k + 1],
                                                in0=mv0[4 * X4 + k][:, 1:2],
                                                scalar1=EPS, scalar2=None, op0=OP.add)
                    rstd0[X4] = self.rsqrt(rs_p, ve[:], 4)
                    for t2 in range(4 * X4, 4 * X4 + 4):
                        xbf = xbf_p.tile([P, D], bf16, tag="xbf", name="xbf")
                        nc.vector.tensor_scalar(out=xbf[:], in0=src_sb[:, t2, :],
                                                scalar1=mv0[t2][:, 0:1],
                                                scalar2=rstd0[X4][:, t2 % 4:t2 % 4 + 1],
                                                op0=OP.subtract, op1=OP.mult)
                        nc.sync.dma_start_transpose(
                            out=hTb[:, :, t2 * P:(t2 + 1) * P], in_=xbf[:])
                if tt == 3:
                    # half-A transposed: ACT converts ib0 while LN0-B runs on DVE
                    for s in range(FC):
                        nc.scalar.activation(out=hT[:, s, 0:512],
                                             in_=hTb[:, s, 0:512], func=AF.Copy)
            with tc.tile_wait_until(1.5):
                late_input_dmas()
            for s in range(FC):
                nc.vector.tensor_copy(hT[:, s, 512:1024], hTb[:, s, 512:1024])

            hTv = hT[:].rearrange("p (g t) n -> p g t n", t=2)
            wqkv = wqk_sb[:].rearrange("p (g t) c -> p g t c", t=2)
            wvv = wv_sb[:].rearrange("p (g t) c -> p g t c", t=2)

            # ---------------- QKV ----------------
            with tc.tile_pool(name="qkps", bufs=3, space="PSUM") as qkps:
                order = [b + 8 * t for b in range(8) for t in range(2)]
                for ocb in order:
                    ps = qkps.tile([P, D], f32, tag="qk", name="psqk")
                    for ib in range(2):
                        for g in range(4):
                            nc.tensor.matmul(ps[:, ib * 512:(ib + 1) * 512],
                                             wqkv[:, g, :, ocb * P:(ocb + 1) * P],
                                             hTv[:, g, :, ib * 512:(ib + 1) * 512],
                                             start=(g == 0), stop=(g == 3),
                                             perf_mode=DR)
                    nc.vector.tensor_scalar(
                        out=qkT[:, ocb, :], in0=ps[:],
                        scalar1=qkb_sb[:, ocb:ocb + 1], scalar2=None, op0=OP.add)

            with tc.tile_pool(name="vps", bufs=2, space="PSUM") as vps:
                for jc in range(TT):
                    ps = vps.tile([P, D], f32, tag="v", name="psv")
                    for vb2 in range(2):
                        for g in range(4):
                            nc.tensor.matmul(ps[:, vb2 * 512:(vb2 + 1) * 512],
                                             hTv[:, g, :, jc * P:(jc + 1) * P],
                                             wvv[:, g, :, vb2 * 512:(vb2 + 1) * 512],
                                             start=(g == 0), stop=(g == 3),
                                             perf_mode=DR)
                    nc.vector.tensor_scalar(
                        out=v_sb[:, jc, :, 0:HD],
                        in0=ps[:].rearrange("p (h c) -> p h c", h=H),
                        scalar1=mq_sb[:, jc:jc + 1], scalar2=None,
                        op0=OP.mult)
                    nc.vector.tensor_copy(
                        v_sb[:, jc, :, HD:HD + 1],
                        mq_sb[:, jc:jc + 1].unsqueeze(2).broadcast_to([P, H, 1]))

            # hbar = mean over tokens of h
            for s in range(FC):
                nc.vector.tensor_reduce(hbar_f[:, s:s + 1], hT[:, s:s + 1, :],
                                        AX.X, OP.add)
            nc.vector.tensor_scalar(out=hbar8[:], in0=hbar_f[:], scalar1=1.0 / N,
                                    scalar2=None, op0=OP.mult)

            self.atokd = {}
            self.attnTd = {}
            self.ptsd = {0: {}, 1: {}}

            # ================== attention + FFN pipeline ==================
            psum = ExitStack()
            s_pool = psum.enter_context(
                tc.tile_pool(name="spool", bufs=2, space="PSUM"))

            def s_head(h, X):
                blk, m2 = h // 2, h % 2
                lo = 64 * m2
                pts = pts_p.tile([P, TT, 512], fp8, tag="pts", name="pts")
                for jp in range(4):
                    ps_s = self.s_pool.tile([P, 2, 512], f32, tag="s", name="ps_s")
                    for sub in range(2):
                        jc = 2 * jp + sub
                        nc.tensor.matmul(
                            ps_s[:, sub, :],
                            qkT[lo:lo + 64, 8 + blk, jc * P:(jc + 1) * P],
                            qkT[lo:lo + 64, blk, X * 512:(X + 1) * 512],
                            start=True, stop=True)
                    nc.scalar.activation(out=pts[:, 2 * jp:2 * jp + 2, :],
                                         in_=ps_s[:], func=AF.Exp, scale=0.125)
                self.ptsd[X][h] = pts

            def av_group(hg2, X):
                for ttl in range(4):
                    tt = X * 4 + ttl
                    if hg2 == 0:
                        self.atokd[tt] = atok_p.tile([P, H, HD], bf16, tag="atok",
                                                     name="atok")
                    ps_av = av_pool.tile([P, 4, P], f32, tag="av", name="ps_av")
                    for hh in range(4):
                        h = 4 * hg2 + hh
                        pts = self.ptsd[X][h]
                        for jp in range(4):
                            nc.tensor.matmul(
                                ps_av[:, hh, 0:HD + 1],
                                pts[:, 2 * jp:2 * jp + 2, ttl * P:(ttl + 1) * P],
                                v_sb[:, 2 * jp:2 * jp + 2, h, :],
                                start=(jp == 0), stop=(jp == 3), perf_mode=DR)
                    rdn = tmp_p.tile([P, 4, 1], f32,
                                     tag="rdnB" if X else "rdn", name="rdn")
                    nc.vector.reciprocal(rdn[:], ps_av[:, :, HD:HD + 1])
                    nc.vector.tensor_tensor(
                        self.atokd[tt][:, 4 * hg2:4 * hg2 + 4, :],
                        ps_av[:, :, 0:HD],
                        rdn[:].broadcast_to([P, 4, HD]), OP.mult)

            def attn_tr(X):
                for tt in range(X * 4, X * 4 + 4):
                    aT = attnT_p.tile([P, FC, P], bf16, tag="attnT", name="attnT")
                    nc.sync.dma_start_transpose(out=aT[:], in_=self.atokd[tt][:])
                    self.attnTd[tt] = aT

            # --- half A: S/exp for heads 0..3 first ---
            for h in range(4):
                s_head(h, 0)

            # --- wb machinery while exp-A streams ---
            with tc.tile_pool(name="wbps", bufs=1, space="PSUM") as wbps:
                ps_vb = wbps.tile([P, FC], f32, tag="vb", name="ps_vb")
                for vblk in range(FC):
                    for s in range(FC):
                        nc.tensor.matmul(ps_vb[:, vblk:vblk + 1],
                                         wv_sb[:, s, vblk * P:(vblk + 1) * P],
                                         hbar8[:, s:s + 1],
                                         start=(s == 0), stop=(s == FC - 1))
                vb_bf = small.tile([P, FC], bf16, tag="vbbf", name="vb_bf")
                nc.vector.tensor_copy(vb_bf[:], ps_vb[:])
                wrow_bf = small.tile([1, D], bf16, tag="wrow", name="wrow_bf")
                for ob2 in range(2):
                    ps_wr = wbps.tile([1, 512], f32, tag="wr", name="ps_wr")
                    for blk in range(FC):
                        nc.tensor.matmul(ps_wr[0:1, :], vb_bf[:, blk:blk + 1],
                                         wproj_sb[:, blk, ob2 * 512:(ob2 + 1) * 512],
                                         start=(blk == 0), stop=(blk == FC - 1))
                    nc.vector.tensor_tensor(
                        wrow_bf[0:1, ob2 * 512:(ob2 + 1) * 512], ps_wr[0:1, :],
                        bprojb[0:1, ob2 * 512:(ob2 + 1) * 512], OP.add)
                for ob2 in range(2):
                    ps_bc = wbps.tile([P, 512], f32, tag="bc", name="ps_bc")
                    nc.tensor.matmul(ps_bc[:], ones_row[:],
                                     wrow_bf[0:1, ob2 * 512:(ob2 + 1) * 512],
                                     start=True, stop=True)
                    nc.vector.tensor_copy(wb_sb[:, ob2 * 512:(ob2 + 1) * 512],
                                          ps_bc[:])

            # srcw: src += invmq*wb + mq*bproj'
            for tt in range(TT):
                nc.vector.scalar_tensor_tensor(
                    out=src_sb[:, tt, :], in0=wb_sb[:], scalar=imq_sb[:, tt:tt + 1],
                    in1=src_sb[:, tt, :], op0=OP.mult, op1=OP.add)
                t2 = tmp_p.tile([P, D], f32, tag="srcw", name="srcw_t", bufs=1)
                nc.gpsimd.tensor_scalar(out=t2[:], in0=bprojb[:],
                                        scalar1=mq_sb[:, tt:tt + 1],
                                        scalar2=None, op0=OP.mult)
                nc.gpsimd.tensor_tensor(src_sb[:, tt, :], src_sb[:, tt, :],
                                        t2[:], OP.add)

            av_pool = psum.enter_context(
                tc.tile_pool(name="avpool", bufs=2, space="PSUM"))
            for h in range(4, 16):
                s_head(h, 0)
                if h % 4 == 3:
                    av_group(h // 4 - 1, 0)
            av_group(3, 0)
            attn_tr(0)
            sA_stack.close()
            sB_stack = ExitStack()
            self.s_pool = sB_stack.enter_context(
                tc.tile_pool(name="spoolB", bufs=2, space="PSUM"))

            # deferred half-B attention units, popped during proj-A / FFN1-A
            bwork = []
            for h in range(16):
                bwork.append(("s", h))
                if h % 4 == 3:
                    bwork.append(("av", h // 4))
            bwork.append(("tr", None))

            def pop_b(k):
                for _ in range(k):
                    if not bwork:
                        return
                    kind, arg = bwork.pop(0)
                    if kind == "s":
                        with tc.tile_wait_until(PAINT * (0.092 + 0.0043 * arg)):
                            s_head(arg, 1)
                    elif kind == "av":
                        with tc.tile_wait_until(PAINT * (0.094 + 0.0172 * (arg + 1))):
                            av_group(arg, 1)
                    else:
                        with tc.tile_wait_until(PAINT * 0.164):
                            attn_tr(1)

            def proj_half(X, proj_pool, pace, wproj_sb=wproj_sb):
                for ttl in range(4):
                    tt = X * 4 + ttl
                    aT = self.attnTd[tt]
                    for ob in range(2):
                        ps_p = proj_pool.tile([P, 512], f32, tag="pj", name="ps_p")
                        for fc in range(FC):
                            nc.tensor.matmul(
                                ps_p[:], aT[:, fc, :],
                                wproj_sb[:, fc, ob * 512:(ob + 1) * 512],
                                start=(fc == 0), stop=(fc == FC - 1))
                        nc.vector.scalar_tensor_tensor(
                            out=src_sb[:, tt, ob * 512:(ob + 1) * 512],
                            in0=ps_p[:], scalar=mq_sb[:, tt:tt + 1],
                            in1=src_sb[:, tt, ob * 512:(ob + 1) * 512],
                            op0=OP.mult, op1=OP.add)
                        if pace:
                            pop_b(1)

            def ln1_half(X, x1nT):
                mvs = []
                for ttl in range(4):
                    mvs.append(self.ln_stats(src_sb[:, X * 4 + ttl, :]))
                ve = rs_p.tile([P, 4], f32, tag="ve", name="ve1")
                for k in range(4):
                    nc.vector.tensor_scalar(out=ve[:, k:k + 1], in0=mvs[k][:, 1:2],
                                            scalar1=EPS, scalar2=None, op0=OP.add)
                rstd = self.rsqrt(rs_p, ve[:], 4)
                for ttl in range(4):
                    tt = X * 4 + ttl
                    xbf1 = xbf_p.tile([P, D], bf16, tag="xbf", name="xbf1")
                    nc.vector.tensor_scalar(out=xbf1[:], in0=src_sb[:, tt, :],
                                            scalar1=mvs[ttl][:, 0:1],
                                            scalar2=rstd[:, ttl:ttl + 1],
                                            op0=OP.subtract, op1=OP.mult)
                    nc.sync.dma_start_transpose(
                        out=x1nT[:, :, ttl * P:(ttl + 1) * P], in_=xbf1[:])
                    self.last_xbf1 = xbf1
                    # x1g = xbf1*g1 + (beta1+b2) into src_sb (Pool, off-critical)
                    nc.gpsimd.tensor_tensor(src_sb[:, tt, :], xbf1[:], g1b, OP.mult)
                    nc.gpsimd.tensor_tensor(src_sb[:, tt, :], src_sb[:, tt, :],
                                            bb2b, OP.add)

            def ffn1_half(X, x1nT, f1ps, pace, drain_dve):
                w1t = {}
                look = 2
                for gc in range(-look, GC):
                    if gc + look < GC:
                        w = self.w1_p.tile([P, D], bf16, tag="w1t", name="w1t")
                        nc.sync.dma_start(out=w[:], in_=w1_h[gc + look, :, :])
                        w1t[gc + look] = w
                    if gc < 0:
                        continue
                    ps_z = f1ps.tile([P, 512], f32, tag="z", name="ps_z")
                    for fc in range(FC):
                        nc.tensor.matmul(ps_z[:], w1t[gc][:, fc * P:(fc + 1) * P],
                                         x1nT[:, fc, :],
                                         start=(fc == 0), stop=(fc == FC - 1))
                    if drain_dve:
                        # +b1' on DVE; gelu applied later in-place (batched),
                        # keeping ACT free for exp-B
                        nc.vector.tensor_scalar(
                            out=self.zT[:, gc, X * 512:(X + 1) * 512],
                            in0=ps_z[:], scalar1=b1p_sb[:, gc:gc + 1],
                            scalar2=None, op0=OP.add)
                    else:
                        nc.scalar.activation(out=self.zT[:, gc, X * 512:(X + 1) * 512],
                                             in_=ps_z[:], func=AF.Gelu,
                                             bias=b1p_sb[:, gc:gc + 1], scale=1.0)
                    del w1t[gc]
                    if pace:
                        pop_b(1)

            def gelu_batch(X, gate_src=None):
                if gate_src is not None:
                    gate = small.tile([P, 1], f32, tag="gelugate", name="gelugate")
                    nc.vector.tensor_scalar(out=gate[:], in0=gate_src,
                                            scalar1=0.0, scalar2=None, op0=OP.mult)
                else:
                    gate = None
                for q in range(8):
                    ap = self.zT[:, 4 * q:4 * q + 4, X * 512:(X + 1) * 512]
                    if gate is None:
                        nc.scalar.activation(out=ap, in_=ap, func=AF.Gelu)
                    else:
                        nc.scalar.activation(out=ap, in_=ap, func=AF.Gelu,
                                             bias=gate[:])

            def ffn2_half(X, f2ps, w2t0):
                w2t = {0: w2t0}
                for ob in range(4):
                    if ob + 1 < 4:
                        w = self.w2_p.tile([P, GC, 256], bf16, tag="w2t", name="w2t")
                        nc.sync.dma_start(out=w[:], in_=w2_h[ob + 1, :, :])
                        w2t[ob + 1] = w
                    for ttl in range(4):
                        tt = X * 4 + ttl
                        ps_y = f2ps.tile([P, 256], f32, tag="y", name="ps_y")
                        for gc in range(GC):
                            nc.tensor.matmul(ps_y[:],
                                             self.zT[:, gc, tt * P:(tt + 1) * P],
                                             w2t[ob][:, gc, :],
                                             start=(gc == 0), stop=(gc == GC - 1))
                        yo = self.yout_p.tile([P, 256], f32, tag="yo", name="yo")
                        nc.vector.tensor_tensor(
                            yo[:], ps_y[:],
                            src_sb[:, tt, ob * 256:(ob + 1) * 256], OP.add)
                        nc.sync.dma_start(
                            out=out_h[tt * P:(tt + 1) * P, ob * 256:(ob + 1) * 256],
                            in_=yo[:])
                    del w2t[ob]

            # --- proj A (S-B interleaved) ---
            with tc.tile_pool(name="pjps", bufs=2, space="PSUM") as pjps:
                proj_half(0, pjps, pace=0)

            right2.close()    # wproj A
            right1.close()    # hTb, hT, wqk, wv

            # right-side pools for LN1/FFN
            with ExitStack() as tail:
                x1nT_p = tail.enter_context(
                    tc.tile_pool(name="x1nTp", bufs=2, side="right"))
                zT_p = tail.enter_context(
                    tc.tile_pool(name="zTp", bufs=1, side="right"))
                self.w1_p = tail.enter_context(
                    tc.tile_pool(name="w1p", bufs=3, side="right"))
                zT = zT_p.tile([P, GC, N], bf16, tag="zT", name="zT")
                self.zT = zT
                x1nT_A = x1nT_p.tile([P, FC, 512], bf16, tag="x1nT", name="x1nT_A")
                ln1_half(0, x1nT_A)
                for _ in range(3):
                    gslot = tmp_p.tile([P, 4, 1], f32, tag="rdnB", name="rdngate")
                    nc.vector.tensor_scalar(
                        out=gslot[:], in0=self.last_xbf1[:, 0:4].unsqueeze(2),
                        scalar1=0.0, scalar2=None, op0=OP.mult)

                # --- FFN1 A with remaining S-B/AV-B interleaved ---
                with tc.tile_pool(name="f1ps", bufs=2, space="PSUM") as f1ps:
                    ffn1_half(0, x1nT_A, f1ps, pace=1, drain_dve=True)
                    pop_b(len(bwork))
                gelu_batch(0, gate_src=self.ptsd[1][15][:, 0, 0:1])

                sB_stack.close()
                psum.close()      # av_pool
                attn_sb.close()   # qkT, v_sb (left)
                pts_stack.close()
                atok_stack.close()

                wpjB_p = tail.enter_context(
                    tc.tile_pool(name="wpjB", bufs=1, side="right"))
                self.w2_p = tail.enter_context(
                    tc.tile_pool(name="w2p", bufs=2, side="right"))
                self.yout_p = tail.enter_context(
                    tc.tile_pool(name="youtp", bufs=4, side="right"))
                wproj_sb = wpjB_p.tile([P, FC, D], bf16)
                nc.sync.dma_start(out=wproj_sb[:], in_=wpj_h[:, :, :])
                w2t0 = self.w2_p.tile([P, GC, 256], bf16, tag="w2t", name="w2t0")
                nc.sync.dma_start(out=w2t0[:], in_=w2_h[0, :, :])

                x1nT_B = x1nT_p.tile([P, FC, 512], bf16, tag="x1nT", name="x1nT_B")
                with tc.tile_pool(name="pjps2", bufs=2, space="PSUM") as pjps2:
                    proj_half(1, pjps2, pace=0, wproj_sb=wproj_sb)
                    ln1_half(1, x1nT_B)
                attnT_stack.close()

                with tc.tile_pool(name="f2aps", bufs=2, space="PSUM") as f2aps:
                    ffn2_half(0, f2aps, w2t0)

                w2t0b = self.w2_p.tile([P, GC, 256], bf16, tag="w2t", name="w2t0b")
                nc.sync.dma_start(out=w2t0b[:], in_=w2_h[0, :, :])
                with tc.tile_pool(name="f1bps", bufs=2, space="PSUM") as f1bps:
                    ffn1_half(1, x1nT_B, f1bps, pace=0, drain_dve=False)
                with tc.tile_pool(name="f2bps", bufs=2, space="PSUM") as f2bps:
                    ffn2_half(1, f2bps, w2t0b)


# ---------------- host side ----------------

_NC_CACHE = {}


def _get_nc():
    if "nc" not in _NC_CACHE:
        _NC_CACHE["nc"] = build_bass()
    return _NC_CACHE["nc"]


def prep_in_maps(inputs):
    e4 = ml_dtypes.float8_e4m3
    bfd = ml_dtypes.bfloat16
    src = np.asarray(inputs["src"], dtype=np.float32)
    mask = np.asarray(inputs["mask"])
    Wqkv = np.asarray(inputs["Wqkv"], dtype=np.float32)
    Wproj = np.asarray(inputs["Wproj"], dtype=np.float32)
    bproj = np.asarray(inputs["bproj"], dtype=np.float32)
    W1 = np.asarray(inputs["W1"], dtype=np.float32)
    b1 = np.asarray(inputs["b1"], dtype=np.float32)
    W2 = np.asarray(inputs["W2"], dtype=np.float32)
    b2 = np.asarray(inputs["b2"], dtype=np.float32)
    g0 = np.asarray(inputs["g0"], dtype=np.float32)
    beta0 = np.asarray(inputs["beta0"], dtype=np.float32)
    g1 = np.asarray(inputs["g1"], dtype=np.float32)
    beta1 = np.asarray(inputs["beta1"], dtype=np.float32)

    Wqkv_s = Wqkv * g0[None, :]
    qkv_bias = Wqkv @ beta0
    Wq, Wk, Wv = Wqkv_s[0:D], Wqkv_s[D:2 * D], Wqkv_s[2 * D:]
    bq, bk, bv = qkv_bias[0:D], qkv_bias[D:2 * D], qkv_bias[2 * D:]
    bprojp = bproj + Wproj @ bv

    WQK = np.concatenate([Wq, Wk], axis=0)                    # [2048, 1024]
    qkb = np.concatenate([bq, bk]).reshape(16, P)

    wqk = np.ascontiguousarray(
        WQK.T.reshape(FC, P, 2 * D).transpose(1, 0, 2)).astype(e4)
    wv8 = np.ascontiguousarray(
        Wv.T.reshape(FC, P, D).transpose(1, 0, 2)).astype(e4)
    wpj = np.ascontiguousarray(
        Wproj.T.reshape(FC, P, D).transpose(1, 0, 2)).astype(bfd)

    W1g = W1 * g1[None, :]
    b1p = (b1 + W1 @ beta1).reshape(GC, P)
    w1 = np.ascontiguousarray(
        W1g.T.reshape(FC, P, GC, P).transpose(2, 1, 0, 3).reshape(GC, P, D)
    ).astype(bfd)
    w2 = np.ascontiguousarray(
        W2.T.reshape(GC, P, 4, 256).transpose(2, 1, 0, 3).reshape(4, P, GC * 256)
    ).astype(bfd)

    vecs = np.ascontiguousarray(np.stack([g1, beta1 + b2, bprojp]))
    mqf = mask.astype(np.float32).reshape(B, TT, P)
    imqf = 1.0 - mqf

    in_maps = []
    for bb in range(B):
        in_maps.append({
            "src": np.ascontiguousarray(src[bb]),
            "mq": np.ascontiguousarray(mqf[bb]),
            "imq": np.ascontiguousarray(imqf[bb]),
            "vecs": vecs,
            "qkb": np.ascontiguousarray(qkb),
            "b1p": np.ascontiguousarray(b1p),
            "wqk": wqk,
            "wv": wv8,
            "wpj": wpj,
            "w1": w1,
            "w2": w2,
        })
    return in_maps


def kernel(**inputs):
    in_maps = prep_in_maps(inputs)
    nc = _get_nc()
    res = run_bass_kernel_spmd(nc, in_maps, core_ids=list(range(B)))
    return np.stack([r["out"] for r in res.results]).astype(np.float32)                with tc.tile_wait_until(1.5):2 NeuronCores, v2.

Data-parallel over batch (1 element/core). Per-core schedule built around the
TimelineSim cost model:
  - fp8e4 DoubleRow matmuls for QKV, S=q.k^T and P@V (0.5 cyc/output row).
  - bf16 matmuls for proj and the FFN (fp8 FFN would bust the 2e-2 gate).
  - token-major P@V (full 128-partition outputs) + per-partition normalize.
  - all transposes via the DMA xbar (dma_start_transpose), none on PE.
  - softmax key-masking by zeroing masked v rows + mask-valued denominator
    column (no per-key bias in exp, enabling wide exp instructions).
  - masked query rows replaced by wb = Wproj @ (Wv' @ mean(h)) + bproj'.
  - LN affine folded into weights on host; rsqrt via bit-trick + Newton on
    DVE so ACT only runs exp / gelu / fp8 converts.
  - query-halves A/B pipelined: FFN(A) overlaps exp(B) on ACT.
"""

import os
import numpy as np
import ml_dtypes

PAINT = 0.0

import concourse.bass as bass
import concourse.tile as tile
from concourse import bacc
from concourse import mybir
from concourse.bass_utils import run_bass_kernel_spmd

B, N, D, H, F = 8, 1024, 1024, 16, 4096
HD = D // H          # 64
P = 128
TT = N // P          # 8 token tiles
FC = D // P          # 8 feature chunks
GC = F // P          # 32 FFN1 chunks
EPS = 1e-5

f32 = mybir.dt.float32
bf16 = mybir.dt.bfloat16
fp8 = mybir.dt.float8e4
i32 = mybir.dt.int32
AF = mybir.ActivationFunctionType
OP = mybir.AluOpType
AX = mybir.AxisListType
DR = mybir.MatmulPerfMode.DoubleRow

MAGIC = 0x5F3759DF


def build_bass():
    nc = bacc.Bacc("TRN2")

    src_h = nc.dram_tensor("src", [N, D], f32, kind="ExternalInput")
    mq_h = nc.dram_tensor("mq", [TT, P], f32, kind="ExternalInput")
    imq_h = nc.dram_tensor("imq", [TT, P], f32, kind="ExternalInput")
    vecs_h = nc.dram_tensor("vecs", [3, D], f32, kind="ExternalInput")
    qkb_h = nc.dram_tensor("qkb", [16, P], f32, kind="ExternalInput")
    b1p_h = nc.dram_tensor("b1p", [GC, P], f32, kind="ExternalInput")
    wqk_h = nc.dram_tensor("wqk", [P, FC, 2 * D], fp8, kind="ExternalInput")
    wv_h = nc.dram_tensor("wv", [P, FC, D], fp8, kind="ExternalInput")
    wpj_h = nc.dram_tensor("wpj", [P, FC, D], bf16, kind="ExternalInput")
    w1_h = nc.dram_tensor("w1", [GC, P, D], bf16, kind="ExternalInput")
    w2_h = nc.dram_tensor("w2", [4, P, GC * 256], bf16, kind="ExternalInput")
    out_h = nc.dram_tensor("out", [N, D], f32, kind="ExternalOutput")

    with tile.TileContext(nc) as tc:
        Builder(nc, tc).run(src_h, mq_h, imq_h, vecs_h, qkb_h, b1p_h,
                            wqk_h, wv_h, wpj_h, w1_h, w2_h, out_h)
    nc.compile()
    return nc


class Builder:
    def __init__(self, nc, tc):
        self.nc = nc
        self.tc = tc

    # ---------- helpers ----------
    def rsqrt(self, pool, ve_ap, n):
        """(ve)^-0.5 elementwise for [128, n] f32 via bit trick + 2 Newton."""
        nc = self.nc
        t = pool.tile([P, n], i32, tag="rsq_t", name="rsq_t")
        nc.vector.tensor_scalar(out=t[:], in0=ve_ap.bitcast(i32), scalar1=1,
                                scalar2=None, op0=OP.arith_shift_right)
        y = pool.tile([P, n], f32, tag="rsq_y", name="rsq_y")
        nc.vector.tensor_scalar(out=y[:].bitcast(i32), in0=t[:], scalar1=-1,
                                scalar2=MAGIC, op0=OP.mult, op1=OP.add)
        for _ in range(2):
            a = pool.tile([P, n], f32, tag="rsq_a", name="rsq_a")
            nc.vector.tensor_tensor(a[:], y[:], y[:], OP.mult)
            nc.vector.tensor_tensor(a[:], a[:], ve_ap, OP.mult)
            nc.vector.tensor_scalar(out=a[:], in0=a[:], scalar1=-0.5,
                                    scalar2=1.5, op0=OP.mult, op1=OP.add)
            nc.vector.tensor_tensor(y[:], y[:], a[:], OP.mult)
        return y

    def ln_stats(self, x_ap):
        nc = self.nc
        st = self.stats_p.tile([P, 2, 6], f32, tag="bst", name="bst")
        for sg in range(2):
            nc.vector.bn_stats(out=st[:, sg, :],
                               in_=x_ap[:, sg * 512:(sg + 1) * 512])
        mv = self.mv_p.tile([P, 2], f32, tag="mv", name="mv")
        nc.vector.bn_aggr(out=mv[:], in_=st[:])
        return mv

    # ---------- main ----------
    def run(self, src_h, mq_h, imq_h, vecs_h, qkb_h, b1p_h,
            wqk_h, wv_h, wpj_h, w1_h, w2_h, out_h):
        nc, tc = self.nc, self.tc
        from contextlib import ExitStack

        with ExitStack() as left:
            consts = left.enter_context(tc.tile_pool(name="consts", bufs=1, side="left"))
            persist = left.enter_context(tc.tile_pool(name="persist", bufs=1, side="left"))
            small = left.enter_context(tc.tile_pool(name="small", bufs=1, side="left"))
            self.stats_p = left.enter_context(tc.tile_pool(name="stats", bufs=3, side="left"))
            self.mv_p = left.enter_context(tc.tile_pool(name="mv", bufs=12, side="left"))
            rs_p = left.enter_context(tc.tile_pool(name="rs", bufs=2, side="left"))
            tmp_p = left.enter_context(tc.tile_pool(name="tmp", bufs=3, side="left"))
            xbf_p = left.enter_context(tc.tile_pool(name="xbf", bufs=2, side="left"))
            attnT_stack = ExitStack()
            attnT_p = attnT_stack.enter_context(
                tc.tile_pool(name="attnTp", bufs=4, side="left"))
            atok_stack = ExitStack()
            atok_p = atok_stack.enter_context(
                tc.tile_pool(name="atokp", bufs=4, side="left"))
            pts_stack = ExitStack()
            pts_p = pts_stack.enter_context(
                tc.tile_pool(name="ptsp", bufs=5, side="left"))

            # ---------------- inputs (src first; weights after LN0) --------
            src_sb = persist.tile([P, TT, D], f32)
            for tt in range(TT):
                nc.sync.dma_start(out=src_sb[:, tt, :],
                                  in_=src_h[tt * P:(tt + 1) * P, :])
            mq_sb = consts.tile([P, TT], f32)
            imq_sb = consts.tile([P, TT], f32)
            qkb_sb = consts.tile([P, 16], f32)
            b1p_sb = consts.tile([P, GC], f32)
            bcast = consts.tile([P, 3, D], f32)
            g1b, bb2b, bprojb = bcast[:, 0], bcast[:, 1], bcast[:, 2]

            ones_row = consts.tile([1, P], bf16)
            nc.vector.memset(ones_row[:], 1.0)
            wb_sb = consts.tile([P, D], f32)
            hbar8 = consts.tile([P, FC], fp8)
            hbar_f = consts.tile([P, FC], f32)

            # right-side scoped pools: early weights + staging
            right1 = ExitStack()
            early = right1.enter_context(
                tc.tile_pool(name="early", bufs=1, side="right"))
            wqk_sb = early.tile([P, FC, 2 * D], fp8)
            wv_sb = early.tile([P, FC, D], fp8)
            hTb = early.tile([P, FC, N], bf16)
            hT = early.tile([P, FC, N], fp8)
            right2 = ExitStack()
            wpjA_p = right2.enter_context(
                tc.tile_pool(name="wpjA", bufs=1, side="right"))
            wproj_sb = wpjA_p.tile([P, FC, D], bf16)

            def late_input_dmas():
                nc.sync.dma_start(out=qkb_sb[:],
                                  in_=qkb_h[:, :].rearrange("a p -> p a"))
                nc.sync.dma_start(out=mq_sb[:],
                                  in_=mq_h[:, :].rearrange("a p -> p a"))
                for g in range(4):
                    nc.sync.dma_start(out=wqk_sb[:, 2 * g:2 * g + 2, :],
                                      in_=wqk_h[:, 2 * g:2 * g + 2, :])
                nc.sync.dma_start(out=wv_sb[:], in_=wv_h[:, :, :])
                nc.sync.dma_start(out=imq_sb[:],
                                  in_=imq_h[:, :].rearrange("a p -> p a"))
                nc.sync.dma_start(out=b1p_sb[:],
                                  in_=b1p_h[:, :].rearrange("a p -> p a"))
                for v3 in range(3):
                    bc_src = bass.AP(tensor=vecs_h[0:1, :].tensor, offset=v3 * D,
                                     ap=[[0, P], [1, D]])
                    nc.sync.dma_start(out=bcast[:, v3, :], in_=bc_src)
                nc.sync.dma_start(out=wproj_sb[:], in_=wpj_h[:, :, :])

            # attention-lifetime left pool (qkT, v) — closes before w2 opens
            attn_sb = ExitStack()
            attn_pool = attn_sb.enter_context(
                tc.tile_pool(name="attnsb", bufs=1, side="left"))
            qkT = attn_pool.tile([P, 16, N], fp8)
            v_sb = attn_pool.tile([P, TT, H, HD + 1], fp8)

            # ---------------- LN0 -> xbf -> hTb -> hT ----------------
            mv0 = []
            rstd0 = [None] * 2
            for tt in range(TT):
                mv0.append(self.ln_stats(src_sb[:, tt, :]))
                if tt % 4 == 3:
                    X4 = tt // 4
                    ve = rs_p.tile([P, 4], f32, tag="ve", name="ve0")
                    for k in range(4):
                        nc.vector.tensor_scalar(out=ve[:, k:k + 1],
                                                in0=mv0[4 * X4 + k][:, 1:2],
                                                scalar1=EPS, scalar2=None, op0=OP.add)
                    rstd0[X4] = self.rsqrt(rs_p, ve[:], 4)
                    for t2 in range(4 * X4, 4 * X4 + 4):
                        xbf = xbf_p.tile([P, D], bf16, tag="xbf", name="xbf")
                        nc.vector.tensor_scalar(out=xbf[:], in0=src_sb[:, t2, :],
                                                scalar1=mv0[t2][:, 0:1],
                                                scalar2=rstd0[X4][:, t2 % 4:t2 % 4 + 1],
                                                op0=OP.subtract, op1=OP.mult)
                        nc.sync.dma_start_transpose(
                            out=hTb[:, :, t2 * P:(t2 + 1) * P], in_=xbf[:])
                if tt == 3:
                    # half-A transposed: ACT converts ib0 while LN0-B runs on DVE
                    for s in range(FC):
                        nc.scalar.activation(out=hT[:, s, 0:512],
                                             in_=hTb[:, s, 0:512], func=AF.Copy)
            with tc.tile_wait_until(1.5):
                late_input_dmas()
            for s in range(FC):
                nc.vector.tensor_copy(hT[:, s, 512:1024], hTb[:, s, 512:1024])

            hTv = hT[:].rearrange("p (g t) n -> p g t n", t=2)
            wqkv = wqk_sb[:].rearrange("p (g t) c -> p g t c", t=2)
            wvv = wv_sb[:].rearrange("p (g t) c -> p g t c", t=2)

            # ---------------- QKV ----------------
            with tc.tile_pool(name="qkps", bufs=3, space="PSUM") as qkps:
                order = [b + 8 * t for b in range(8) for t in range(2)]
                for ocb in order:
                    ps = qkps.tile([P, D], f32, tag="qk", name="psqk")
                    for ib in range(2):
                        for g in range(4):
                            nc.tensor.matmul(ps[:, ib * 512:(ib + 1) * 512],
                                             wqkv[:, g, :, ocb * P:(ocb + 1) * P],
                                             hTv[:, g, :, ib * 512:(ib + 1) * 512],
                                             start=(g == 0), stop=(g == 3),
                                             perf_mode=DR)
                    nc.vector.tensor_scalar(
                        out=qkT[:, ocb, :], in0=ps[:],
                        scalar1=qkb_sb[:, ocb:ocb + 1], scalar2=None, op0=OP.add)

            with tc.tile_pool(name="vps", bufs=2, space="PSUM") as vps:
                for jc in range(TT):
                    ps = vps.tile([P, D], f32, tag="v", name="psv")
                    for vb2 in range(2):
                        for g in range(4):
                            nc.tensor.matmul(ps[:, vb2 * 512:(vb2 + 1) * 512],
                                             hTv[:, g, :, jc * P:(jc + 1) * P],
                                             wvv[:, g, :, vb2 * 512:(vb2 + 1) * 512],
                                             start=(g == 0), stop=(g == 3),
                                             perf_mode=DR)
                    nc.vector.tensor_scalar(
                        out=v_sb[:, jc, :, 0:HD],
                        in0=ps[:].rearrange("p (h c) -> p h c", h=H),
                        scalar1=mq_sb[:, jc:jc + 1], scalar2=None,
                        op0=OP.mult)
                    nc.vector.tensor_copy(
                        v_sb[:, jc, :, HD:HD + 1],
                        mq_sb[:, jc:jc + 1].unsqueeze(2).broadcast_to([P, H, 1]))

            # hbar = mean over tokens of h
            for s in range(FC):
                nc.vector.tensor_reduce(hbar_f[:, s:s + 1], hT[:, s:s + 1, :],
                                        AX.X, OP.add)
            nc.vector.tensor_scalar(out=hbar8[:], in0=hbar_f[:], scalar1=1.0 / N,
                                    scalar2=None, op0=OP.mult)

            self.atokd = {}
            self.attnTd = {}
            self.ptsd = {0: {}, 1: {}}

            # ================== attention + FFN pipeline ==================
            psum = ExitStack()
            s_pool = psum.enter_context(
                tc.tile_pool(name="spool", bufs=2, space="PSUM"))

            def s_head(h, X):
                blk, m2 = h // 2, h % 2
                lo = 64 * m2
                pts = pts_p.tile([P, TT, 512], fp8, tag="pts", name="pts")
                for jp in range(4):
                    ps_s = self.s_pool.tile([P, 2, 512], f32, tag="s", name="ps_s")
                    for sub in range(2):
                        jc = 2 * jp + sub
                        nc.tensor.matmul(
                            ps_s[:, sub, :],
                            qkT[lo:lo + 64, 8 + blk, jc * P:(jc + 1) * P],
                            qkT[lo:lo + 64, blk, X * 512:(X + 1) * 512],
                            start=True, stop=True)
                    nc.scalar.activation(out=pts[:, 2 * jp:2 * jp + 2, :],
                                         in_=ps_s[:], func=AF.Exp, scale=0.125)
                self.ptsd[X][h] = pts

            def av_group(hg2, X):
                for ttl in range(4):
                    tt = X * 4 + ttl
                    if hg2 == 0:
                        self.atokd[tt] = atok_p.tile([P, H, HD], bf16, tag="atok",
                                                     name="atok")
                    ps_av = av_pool.tile([P, 4, P], f32, tag="av", name="ps_av")
                    for hh in range(4):
                        h = 4 * hg2 + hh
                        pts = self.ptsd[X][h]
                        for jp in range(4):
                            nc.tensor.matmul(
                                ps_av[:, hh, 0:HD + 1],
                                pts[:, 2 * jp:2 * jp + 2, ttl * P:(ttl + 1) * P],
                                v_sb[:, 2 * jp:2 * jp + 2, h, :],
                                start=(jp == 0), stop=(jp == 3), perf_mode=DR)
                    rdn = tmp_p.tile([P, 4, 1], f32,
                                     tag="rdnB" if X else "rdn", name="rdn")
                    nc.vector.reciprocal(rdn[:], ps_av[:, :, HD:HD + 1])
                    nc.vector.tensor_tensor(
                        self.atokd[tt][:, 4 * hg2:4 * hg2 + 4, :],
                        ps_av[:, :, 0:HD],
                        rdn[:].broadcast_to([P, 4, HD]), OP.mult)

            def attn_tr(X):
                for tt in range(X * 4, X * 4 + 4):
                    aT = attnT_p.tile([P, FC, P], bf16, tag="attnT", name="attnT")
                    nc.sync.dma_start_transpose(out=aT[:], in_=self.atokd[tt][:])
                    self.attnTd[tt] = aT

            # --- half A: S/exp for heads 0..3 first ---
            for h in range(4):
                s_head(h, 0)

            # --- wb machinery while exp-A streams ---
            with tc.tile_pool(name="wbps", bufs=1, space="PSUM") as wbps:
                ps_vb = wbps.tile([P, FC], f32, tag="vb", name="ps_vb")
                for vblk in range(FC):
                    for s in range(FC):
                        nc.tensor.matmul(ps_vb[:, vblk:vblk + 1],
                                         wv_sb[:, s, vblk * P:(vblk + 1) * P],
                                         hbar8[:, s:s + 1],
                                         start=(s == 0), stop=(s == FC - 1))
                vb_bf = small.tile([P, FC], bf16, tag="vbbf", name="vb_bf")
                nc.vector.tensor_copy(vb_bf[:], ps_vb[:])
                wrow_bf = small.tile([1, D], bf16, tag="wrow", name="wrow_bf")
                for ob2 in range(2):
                    ps_wr = wbps.tile([1, 512], f32, tag="wr", name="ps_wr")
                    for blk in range(FC):
                        nc.tensor.matmul(ps_wr[0:1, :], vb_bf[:, blk:blk + 1],
                                         wproj_sb[:, blk, ob2 * 512:(ob2 + 1) * 512],
                                         start=(blk == 0), stop=(blk == FC - 1))
                    nc.vector.tensor_tensor(
                        wrow_bf[0:1, ob2 * 512:(ob2 + 1) * 512], ps_wr[0:1, :],
                        bprojb[0:1, ob2 * 512:(ob2 + 1) * 512], OP.add)
                for ob2 in range(2):
                    ps_bc = wbps.tile([P, 512], f32, tag="bc", name="ps_bc")
                    nc.tensor.matmul(ps_bc[:], ones_row[:],
                                     wrow_bf[0:1, ob2 * 512:(ob2 + 1) * 512],
                                     start=True, stop=True)
                    nc.vector.tensor_copy(wb_sb[:, ob2 * 512:(ob2 + 1) * 512],
                                          ps_bc[:])

            # srcw: src += invmq*wb + mq*bproj'
            for tt in range(TT):
                nc.vector.scalar_tensor_tensor(
                    out=src_sb[:, tt, :], in0=wb_sb[:], scalar=imq_sb[:, tt:tt + 1],
                    in1=src_sb[:, tt, :], op0=OP.mult, op1=OP.add)
                t2 = tmp_p.tile([P, D], f32, tag="srcw", name="srcw_t", bufs=1)
                nc.gpsimd.tensor_scalar(out=t2[:], in0=bprojb[:],
                                        scalar1=mq_sb[:, tt:tt + 1],
                                        scalar2=None, op0=OP.mult)
                nc.gpsimd.tensor_tensor(src_sb[:, tt, :], src_sb[:, tt, :],
                                        t2[:], OP.add)

            av_pool = psum.enter_context(
                tc.tile_pool(name="avpool", bufs=2, space="PSUM"))
            for h in range(4, 16):
                s_head(h, 0)
                if h % 4 == 3:
                    av_group(h // 4 - 1, 0)
            av_group(3, 0)
            attn_tr(0)
            sA_stack.close()
            sB_stack = ExitStack()
            self.s_pool = sB_stack.enter_context(
                tc.tile_pool(name="spoolB", bufs=2, space="PSUM"))

            # deferred half-B attention units, popped during proj-A / FFN1-A
            bwork = []
            for h in range(16):
                bwork.append(("s", h))
                if h % 4 == 3:
                    bwork.append(("av", h // 4))
            bwork.append(("tr", None))

            def pop_b(k):
                for _ in range(k):
                    if not bwork:
                        return
                    kind, arg = bwork.pop(0)
                    if kind == "s":
                        with tc.tile_wait_until(PAINT * (0.092 + 0.0043 * arg)):
                            s_head(arg, 1)
                    elif kind == "av":
                        with tc.tile_wait_until(PAINT * (0.094 + 0.0172 * (arg + 1))):
                            av_group(arg, 1)
                    else:
                        with tc.tile_wait_until(PAINT * 0.164):
                            attn_tr(1)

            def proj_half(X, proj_pool, pace, wproj_sb=wproj_sb):
                for ttl in range(4):
                    tt = X * 4 + ttl
                    aT = self.attnTd[tt]
                    for ob in range(2):
                        ps_p = proj_pool.tile([P, 512], f32, tag="pj", name="ps_p")
                        for fc in range(FC):
                            nc.tensor.matmul(
                                ps_p[:], aT[:, fc, :],
                                wproj_sb[:, fc, ob * 512:(ob + 1) * 512],
                                start=(fc == 0), stop=(fc == FC - 1))
                        nc.vector.scalar_tensor_tensor(
                            out=src_sb[:, tt, ob * 512:(ob + 1) * 512],
                            in0=ps_p[:], scalar=mq_sb[:, tt:tt + 1],
                            in1=src_sb[:, tt, ob * 512:(ob + 1) * 512],
                            op0=OP.mult, op1=OP.add)
                        if pace:
                            pop_b(1)

            def ln1_half(X, x1nT):
                mvs = []
                for ttl in range(4):
                    mvs.append(self.ln_stats(src_sb[:, X * 4 + ttl, :]))
                ve = rs_p.tile([P, 4], f32, tag="ve", name="ve1")
                for k in range(4):
                    nc.vector.tensor_scalar(out=ve[:, k:k + 1], in0=mvs[k][:, 1:2],
                                            scalar1=EPS, scalar2=None, op0=OP.add)
                rstd = self.rsqrt(rs_p, ve[:], 4)
                for ttl in range(4):
                    tt = X * 4 + ttl
                    xbf1 = xbf_p.tile([P, D], bf16, tag="xbf", name="xbf1")
                    nc.vector.tensor_scalar(out=xbf1[:], in0=src_sb[:, tt, :],
                                            scalar1=mvs[ttl][:, 0:1],
                                            scalar2=rstd[:, ttl:ttl + 1],
                                            op0=OP.subtract, op1=OP.mult)
                    nc.sync.dma_start_transpose(
                        out=x1nT[:, :, ttl * P:(ttl + 1) * P], in_=xbf1[:])
                    self.last_xbf1 = xbf1
                    # x1g = xbf1*g1 + (beta1+b2) into src_sb (Pool, off-critical)
                    nc.gpsimd.tensor_tensor(src_sb[:, tt, :], xbf1[:], g1b, OP.mult)
                    nc.gpsimd.tensor_tensor(src_sb[:, tt, :], src_sb[:, tt, :],
                                            bb2b, OP.add)

            def ffn1_half(X, x1nT, f1ps, pace, drain_dve):
                w1t = {}
                look = 2
                for gc in range(-look, GC):
                    if gc + look < GC:
                        w = self.w1_p.tile([P, D], bf16, tag="w1t", name="w1t")
                        nc.sync.dma_start(out=w[:], in_=w1_h[gc + look, :, :])
                        w1t[gc + look] = w
                    if gc < 0:
                        continue
                    ps_z = f1ps.tile([P, 512], f32, tag="z", name="ps_z")
                    for fc in range(FC):
                        nc.tensor.matmul(ps_z[:], w1t[gc][:, fc * P:(fc + 1) * P],
                                         x1nT[:, fc, :],
                                         start=(fc == 0), stop=(fc == FC - 1))
                    if drain_dve:
                        # +b1' on DVE; gelu applied later in-place (batched),
                        # keeping ACT free for exp-B
                        nc.vector.tensor_scalar(
                            out=self.zT[:, gc, X * 512:(X + 1) * 512],
                            in0=ps_z[:], scalar1=b1p_sb[:, gc:gc + 1],
                            scalar2=None, op0=OP.add)
                    else:
                        nc.scalar.activation(out=self.zT[:, gc, X * 512:(X + 1) * 512],
                                             in_=ps_z[:], func=AF.Gelu,
                                             bias=b1p_sb[:, gc:gc + 1], scale=1.0)
                    del w1t[gc]
                    if pace:
                        pop_b(1)

            def gelu_batch(X, gate_src=None):
                if gate_src is not None:
                    gate = small.tile([P, 1], f32, tag="gelugate", name="gelugate")
                    nc.vector.tensor_scalar(out=gate[:], in0=gate_src,
                                            scalar1=0.0, scalar2=None, op0=OP.mult)
                else:
                    gate = None
                for q in range(8):
                    ap = self.zT[:, 4 * q:4 * q + 4, X * 512:(X + 1) * 512]
                    if gate is None:
                        nc.scalar.activation(out=ap, in_=ap, func=AF.Gelu)
                    else:
                        nc.scalar.activation(out=ap, in_=ap, func=AF.Gelu,
                                             bias=gate[:])

            def ffn2_half(X, f2ps, w2t0):
                w2t = {0: w2t0}
                for ob in range(4):
                    if ob + 1 < 4:
                        w = self.w2_p.tile([P, GC, 256], bf16, tag="w2t", name="w2t")
                        nc.sync.dma_start(out=w[:], in_=w2_h[ob + 1, :, :])
                        w2t[ob + 1] = w
                    for ttl in range(4):
                        tt = X * 4 + ttl
                        ps_y = f2ps.tile([P, 256], f32, tag="y", name="ps_y")
                        for gc in range(GC):
                            nc.tensor.matmul(ps_y[:],
                                             self.zT[:, gc, tt * P:(tt + 1) * P],
                                             w2t[ob][:, gc, :],
                                             start=(gc == 0), stop=(gc == GC - 1))
                        yo = self.yout_p.tile([P, 256], f32, tag="yo", name="yo")
                        nc.vector.tensor_tensor(
                            yo[:], ps_y[:],
                            src_sb[:, tt, ob * 256:(ob + 1) * 256], OP.add)
                        nc.sync.dma_start(
                            out=out_h[tt * P:(tt + 1) * P, ob * 256:(ob + 1) * 256],
                            in_=yo[:])
                    del w2t[ob]

            # --- proj A (S-B interleaved) ---
            with tc.tile_pool(name="pjps", bufs=2, space="PSUM") as pjps:
                proj_half(0, pjps, pace=0)

            right2.close()    # wproj A
            right1.close()    # hTb, hT, wqk, wv

            # right-side pools for LN1/FFN
            with ExitStack() as tail:
                x1nT_p = tail.enter_context(
                    tc.tile_pool(name="x1nTp", bufs=2, side="right"))
                zT_p = tail.enter_context(
                    tc.tile_pool(name="zTp", bufs=1, side="right"))
                self.w1_p = tail.enter_context(
                    tc.tile_pool(name="w1p", bufs=3, side="right"))
                zT = zT_p.tile([P, GC, N], bf16, tag="zT", name="zT")
                self.zT = zT
                x1nT_A = x1nT_p.tile([P, FC, 512], bf16, tag="x1nT", name="x1nT_A")
                ln1_half(0, x1nT_A)
                for _ in range(3):
                    gslot = tmp_p.tile([P, 4, 1], f32, tag="rdnB", name="rdngate")
                    nc.vector.tensor_scalar(
                        out=gslot[:], in0=self.last_xbf1[:, 0:4].unsqueeze(2),
                        scalar1=0.0, scalar2=None, op0=OP.mult)

                # --- FFN1 A with remaining S-B/AV-B interleaved ---
                with tc.tile_pool(name="f1ps", bufs=2, space="PSUM") as f1ps:
                    ffn1_half(0, x1nT_A, f1ps, pace=1, drain_dve=True)
                    pop_b(len(bwork))
                gelu_batch(0, gate_src=self.ptsd[1][15][:, 0, 0:1])

                sB_stack.close()
                psum.close()      # av_pool
                attn_sb.close()   # qkT, v_sb (left)
                pts_stack.close()
                atok_stack.close()

                wpjB_p = tail.enter_context(
                    tc.tile_pool(name="wpjB", bufs=1, side="right"))
                self.w2_p = tail.enter_context(
                    tc.tile_pool(name="w2p", bufs=2, side="right"))
                self.yout_p = tail.enter_context(
                    tc.tile_pool(name="youtp", bufs=4, side="right"))
                wproj_sb = wpjB_p.tile([P, FC, D], bf16)
                nc.sync.dma_start(out=wproj_sb[:], in_=wpj_h[:, :, :])
                w2t0 = self.w2_p.tile([P, GC, 256], bf16, tag="w2t", name="w2t0")
                nc.sync.dma_start(out=w2t0[:], in_=w2_h[0, :, :])

                x1nT_B = x1nT_p.tile([P, FC, 512], bf16, tag="x1nT", name="x1nT_B")
                with tc.tile_pool(name="pjps2", bufs=2, space="PSUM") as pjps2:
                    proj_half(1, pjps2, pace=0, wproj_sb=wproj_sb)
                    ln1_half(1, x1nT_B)
                attnT_stack.close()

                with tc.tile_pool(name="f2aps", bufs=2, space="PSUM") as f2aps:
                    ffn2_half(0, f2aps, w2t0)

                w2t0b = self.w2_p.tile([P, GC, 256], bf16, tag="w2t", name="w2t0b")
                nc.sync.dma_start(out=w2t0b[:], in_=w2_h[0, :, :])
                with tc.tile_pool(name="f1bps", bufs=2, space="PSUM") as f1bps:
                    ffn1_half(1, x1nT_B, f1bps, pace=0, drain_dve=False)
                with tc.tile_pool(name="f2bps", bufs=2, space="PSUM") as f2bps:
                    ffn2_half(1, f2bps, w2t0b)


# ---------------- host side ----------------

_NC_CACHE = {}


def _get_nc():
    if "nc" not in _NC_CACHE:
        _NC_CACHE["nc"] = build_bass()
    return _NC_CACHE["nc"]


def prep_in_maps(inputs):
    e4 = ml_dtypes.float8_e4m3
    bfd = ml_dtypes.bfloat16
    src = np.asarray(inputs["src"], dtype=np.float32)
    mask = np.asarray(inputs["mask"])
    Wqkv = np.asarray(inputs["Wqkv"], dtype=np.float32)
    Wproj = np.asarray(inputs["Wproj"], dtype=np.float32)
    bproj = np.asarray(inputs["bproj"], dtype=np.float32)
    W1 = np.asarray(inputs["W1"], dtype=np.float32)
    b1 = np.asarray(inputs["b1"], dtype=np.float32)
    W2 = np.asarray(inputs["W2"], dtype=np.float32)
    b2 = np.asarray(inputs["b2"], dtype=np.float32)
    g0 = np.asarray(inputs["g0"], dtype=np.float32)
    beta0 = np.asarray(inputs["beta0"], dtype=np.float32)
    g1 = np.asarray(inputs["g1"], dtype=np.float32)
    beta1 = np.asarray(inputs["beta1"], dtype=np.float32)

    Wqkv_s = Wqkv * g0[None, :]
    qkv_bias = Wqkv @ beta0
    Wq, Wk, Wv = Wqkv_s[0:D], Wqkv_s[D:2 * D], Wqkv_s[2 * D:]
    bq, bk, bv = qkv_bias[0:D], qkv_bias[D:2 * D], qkv_bias[2 * D:]
    bprojp = bproj + Wproj @ bv

    WQK = np.concatenate([Wq, Wk], axis=0)                    # [2048, 1024]
    qkb = np.concatenate([bq, bk]).reshape(16, P)

    wqk = np.ascontiguousarray(
        WQK.T.reshape(FC, P, 2 * D).transpose(1, 0, 2)).astype(e4)
    wv8 = np.ascontiguousarray(
        Wv.T.reshape(FC, P, D).transpose(1, 0, 2)).astype(e4)
    wpj = np.ascontiguousarray(
        Wproj.T.reshape(FC, P, D).transpose(1, 0, 2)).astype(bfd)

    W1g = W1 * g1[None, :]
    b1p = (b1 + W1 @ beta1).reshape(GC, P)
    w1 = np.ascontiguousarray(
        W1g.T.reshape(FC, P, GC, P).transpose(2, 1, 0, 3).reshape(GC, P, D)
    ).astype(bfd)
    w2 = np.ascontiguousarray(
        W2.T.reshape(GC, P, 4, 256).transpose(2, 1, 0, 3).reshape(4, P, GC * 256)
    ).astype(bfd)

    vecs = np.ascontiguousarray(np.stack([g1, beta1 + b2, bprojp]))
    mqf = mask.astype(np.float32).reshape(B, TT, P)
    imqf = 1.0 - mqf

    in_maps = []
    for bb in range(B):
        in_maps.append({
            "src": np.ascontiguousarray(src[bb]),
            "mq": np.ascontiguousarray(mqf[bb]),
            "imq": np.ascontiguousarray(imqf[bb]),
            "vecs": vecs,
            "qkb": np.ascontiguousarray(qkb),
            "b1p": np.ascontiguousarray(b1p),
            "wqk": wqk,
            "wv": wv8,
            "wpj": wpj,
            "w1": w1,
            "w2": w2,
        })
    return in_maps


def kernel(**inputs):
    in_maps = prep_in_maps(inputs)
    nc = _get_nc()
    res = run_bass_kernel_spmd(nc, in_maps, core_ids=list(range(B)))
    return np.stack([r["out"] for r in res.results]).astype(np.float32)
